# revision 17
# baseline (speedup 1.0000x reference)
"""Trainium2 Bass kernel for nn_AttentionBlock_223338299515.

Reference (B=4, C=128, H=W=64, N=4096 tokens, 4 heads, d_k=32):
  xs = x.reshape(B,C,N).T ; qkv = xs @ Wp.T + bp ; q,k,v = split(qkv)
  attn = softmax_over_queries(q k^T / sqrt(dk)) ; res = attn-weighted v
  out = (res @ Wo.T + bo + xs).T -> (B, C, H, W)

8 NeuronCores, SPMD: core = (batch b = core//2, head pair hp = core%2).
All math in channel-transposed layout (x[b] used directly as (C, N)):
  S^T[j,i] = x_j^T (Wq^T Wk) x_i = sum_c G[c,j] x[c,i],  G = M^T x
    (M = Wq_h^T Wk_h is folded on the host, so Q/K never materialize:
     one G projection per head and x itself is the S-matmul moving side)
  P[j,i] = exp(S^T*scale - c)  (fp8e4; global shift c cancels in P/Z)
  Z[j]   = sum_i P[j,i]
  U[j,c] = (V^T.T @ WoT_h) * 4096/Z[j]   (fp8e4, merged attn-out+out-proj)
  out^T[c,i] = sum_{h,j} U[j,c] P[j,i] / 4096  (+ gamma*x residual)
Host: out[b] = core(2b) + core(2b+1) + bo.

qkv-bias folding: S^T gains (u.x_i) + (w.x_j + bq.bk) with u = Wk^T bq,
w = Wq^T bk.  The per-j part is constant along the softmax axis (i) and
cancels in P/Z, so only u survives — added per-partition during the
G-copy.  v-bias is added per-partition during the V-copy.  So one kernel
handles any bp.

The exp over the 33.5M-element S matrix is the bottleneck: ACT computes
most chunks natively to fp8 (accum_out gives Z for free); a share is
offloaded to DVE as a Schraudolph bit-trick exp (round(s*8/ln2 + B)
saturated to uint8, bitcast fp8e4) plus a DVE copy-with-accumulate for
those chunks' Z partials.  GPSIMD cannot touch PSUM, so it only does the
residual init.  Out-matmuls are fp8 DoubleRow (K=256, 0.5 cyc/col) over
strip pairs, accumulating 4 superblocks per PSUM tile before one DVE
read-modify-write into out_acc.
NOTE: HW fp8e4 encodes inf/NaN at exponent 15 (max normal 240, unlike
e4m3fn's 448) — the shift c keeps every fp8 value below 240.
"""
import os
import sys

import numpy as np

for _p in ("/opt/trn_rl_repo", "/root/.axon_site/_ro/trn_rl_repo"):
    if os.path.isdir(_p) and _p not in sys.path:
        sys.path.insert(0, _p)

import concourse.bacc as bacc
import concourse.tile as tile
from concourse import mybir
from concourse import bass_utils

F32 = mybir.dt.float32
F32R = mybir.dt.float32r
BF16 = mybir.dt.bfloat16
FP8 = mybir.dt.float8e4
U8 = mybir.dt.uint8
EXP = mybir.ActivationFunctionType.Exp
ADD = mybir.AluOpType.add
MULT = mybir.AluOpType.mult
DR = mybir.MatmulPerfMode.DoubleRow

N = 4096
C = 128
DK = 32
SCALE = float(DK) ** -0.5
NCH = 4            # 1024-col i-chunks per strip
NIC = 8            # 512-col out chunks
WSB = 4            # superblocks per window
NW = 8 // WSB      # windows per head
WST = 4 * WSB      # strips per window

CSHIFT = 1.77      # global logit shift; cancels in P/Z, keeps fp8 < 240
LN2 = float(np.log(2.0))
A8 = 8.0 / LN2
SC8 = SCALE * A8                              # Schraudolph scale on raw S
B8 = (7.0 - 0.0579) * 8.0 - CSHIFT * A8       # Schraudolph bias
USCALE = 4096.0

# exp engine schedule: True -> chunk offloaded to DVE Schraudolph (plus a
# DVE copy-with-accumulate for its Z partial). Bresenham over a 64-chunk
# period, interleaved so ACT and DVE consume the st ring concurrently.
NUM_O = 21
PAT_O = [((u * NUM_O) % 64) < NUM_O for u in range(64)]


def build_kernel():
    nc = bacc.Bacc("TRN2", target_bir_lowering=False, debug=False)

    xb_d = nc.dram_tensor("xb", (C, N), BF16, kind="ExternalInput")
    wproj_d = nc.dram_tensor("wproj", (C, 320), BF16, kind="ExternalInput")
    wot_d = nc.dram_tensor("wot", (C, 256), BF16, kind="ExternalInput")
    ub_d = nc.dram_tensor("ub", (C, 2), F32, kind="ExternalInput")
    vb_d = nc.dram_tensor("vb", (C, 2), F32, kind="ExternalInput")
    gamma_d = nc.dram_tensor("gamma", (C, 1), F32, kind="ExternalInput")
    out_d = nc.dram_tensor("out", (C, N), F32, kind="ExternalOutput")

    with tile.TileContext(nc) as tc:
        with (
            tc.tile_pool(name="const", bufs=1) as cpool,
            tc.tile_pool(name="gt", bufs=2) as gtp,
            tc.tile_pool(name="vt", bufs=2) as vtp,
            tc.tile_pool(name="pw", bufs=2) as ppool,
            tc.tile_pool(name="upair", bufs=16) as upool,
            tc.tile_pool(name="zp", bufs=6) as zpp,
            tc.tile_pool(name="zs", bufs=6) as zsp,
            tc.tile_pool(name="zr", bufs=8) as zrp,
            tc.tile_pool(name="scr", bufs=2) as scrp,
            tc.tile_pool(name="st", bufs=3, space="PSUM") as stp,
            tc.tile_pool(name="po", bufs=2, space="PSUM") as pso,
        ):
            xb = cpool.tile([C, N], BF16)
            for dc in range(8):
                dsl = slice(512 * dc, 512 * (dc + 1))
                nc.sync.dma_start(out=xb[:, dsl], in_=xb_d.ap()[:, dsl])
            wproj = cpool.tile([C, 320], BF16)
            nc.sync.dma_start(out=wproj[:], in_=wproj_d.ap())
            wot = cpool.tile([C, 256], BF16)
            nc.sync.dma_start(out=wot[:], in_=wot_d.ap())
            ub = cpool.tile([C, 2], F32)
            nc.sync.dma_start(out=ub[:], in_=ub_d.ap())
            vb = cpool.tile([C, 2], F32)
            nc.sync.dma_start(out=vb[:], in_=vb_d.ap())
            gamma = cpool.tile([C, 1], F32)
            nc.sync.dma_start(out=gamma[:], in_=gamma_d.ap())
            bias_t = cpool.tile([C, 1], F32)
            nc.vector.memset(bias_t[:], -CSHIFT)
            out_acc = cpool.tile([C, N], F32)

            # residual init: out_acc = gamma * x (gamma is 1 or 0)
            for dc in range(8):
                dsl = slice(512 * dc, 512 * (dc + 1))
                nc.vector.tensor_scalar(
                    out=out_acc[:, dsl], in0=xb[:, dsl],
                    scalar1=gamma[:], scalar2=None, op0=MULT)

            pending = []
            pending_u = []

            def drain(k=1):
                for _ in range(k):
                    if pending:
                        pending.pop(0)()

            def drain_u(k=1):
                for _ in range(k):
                    if pending_u:
                        pending_u.pop(0)()

            def alloc_gv(h):
                GT = gtp.tile([C, N], BF16, name=f"GT{h}", tag="GT")
                # V packed on partition quadrants: rows 32q..32q+31 hold
                # v-cols 1024q..1024q+1023 (wot is replicated to match).
                VT = vtp.tile([C, N // 4], BF16, name=f"VT{h}", tag="VT")
                return GT, VT

            def emit_gproj(h, gv, dc):
                GT, _ = gv
                pj = stp.tile([C, 1024], F32, name="pj", tag="st")
                csl = slice(1024 * dc, 1024 * (dc + 1))
                for half in range(2):
                    xsl = slice(1024 * dc + 512 * half,
                                1024 * dc + 512 * (half + 1))
                    nc.tensor.matmul(
                        pj[:, 512 * half: 512 * (half + 1)],
                        wproj[:, 160 * h: 160 * h + 128], xb[:, xsl],
                        start=True, stop=True)
                nc.scalar.activation(
                    out=GT[:, csl], in_=pj[:],
                    func=mybir.ActivationFunctionType.Identity,
                    scale=1.0, bias=ub[:, h: h + 1])

            def emit_vproj(h, gv):
                _, VT = gv
                pj = stp.tile([C, 1024], F32, name="pj", tag="st")
                for q in range(4):
                    for half in range(2):
                        xsl = slice(1024 * q + 512 * half,
                                    1024 * q + 512 * (half + 1))
                        nc.tensor.matmul(
                            pj[32 * q: 32 * (q + 1),
                               512 * half: 512 * (half + 1)],
                            wproj[:, 160 * h + 128: 160 * h + 160],
                            xb[:, xsl],
                            start=True, stop=True,
                            tile_position=(0, 32 * q))
                nc.scalar.activation(
                    out=VT[:], in_=pj[:],
                    func=mybir.ActivationFunctionType.Identity,
                    scale=1.0, bias=vb[:, h: h + 1])

            def emit_out_group(P, Upairs, ic, final):
                isl = slice(512 * ic, 512 * (ic + 1))
                op = pso.tile([C, 512], F32, name="op", tag="po")
                npair = len(Upairs)
                for t in range(npair):
                    nc.tensor.matmul(
                        op[:], Upairs[t][:], P[:, 2 * t: 2 * t + 2, isl],
                        start=(t == 0), stop=(t == npair - 1), perf_mode=DR)
                nc.vector.scalar_tensor_tensor(
                    out=out_acc[:, isl], in0=op[:], scalar=1.0 / USCALE,
                    in1=out_acc[:, isl], op0=MULT, op1=ADD)
                if final:
                    nc.sync.dma_start(out=out_d.ap()[:, isl],
                                      in_=out_acc[:, isl])

            cur_gv = alloc_gv(0)
            next_gv = None
            emit_gproj(0, cur_gv, 0)
            emit_vproj(0, cur_gv)

            for h in range(2):
                GT, VT = cur_gv
                wplan = [4, 4] if h == 0 else [4, 2, 2]
                wbase = 0
                for w, wsb in enumerate(wplan):
                    wst = 4 * wsb
                    P = ppool.tile([C, wst, N], FP8, name=f"P{w % 2}",
                                   tag="P")
                    Upairs = [upool.tile([C, 2, C], FP8, name=f"U{t}",
                                         tag="U") for t in range(wst // 2)]
                    for row in range(wst):
                        sidx = wbase + row
                        jsl = slice(128 * sidx, 128 * (sidx + 1))
                        zparts = zpp.tile([C, NCH], F32, name="zparts")
                        for cch in range(NCH):
                            unit = sidx * NCH + cch
                            if unit in (26, 58, 90):
                                emit_gproj(h, cur_gv, (unit + 6) // 32)
                            elif h == 0 and unit == 104:
                                next_gv = alloc_gv(1)
                                emit_gproj(1, next_gv, 0)
                            elif h == 0 and unit == 116:
                                emit_vproj(1, next_gv)
                            csl = slice(1024 * cch, 1024 * (cch + 1))
                            st = stp.tile([C, 1024], F32, name="st", tag="st")
                            for half in range(2):
                                xsl = slice(1024 * cch + 512 * half,
                                            1024 * cch + 512 * (half + 1))
                                nc.tensor.matmul(
                                    st[:, 512 * half: 512 * (half + 1)],
                                    GT[:, jsl], xb[:, xsl],
                                    start=True, stop=True)
                            zslot = zparts[:, cch: cch + 1]
                            if PAT_O[unit % 64]:
                                nc.vector.tensor_scalar(
                                    out=P[:, row, csl].bitcast(U8),
                                    in0=st[:], scalar1=SC8, scalar2=B8,
                                    op0=MULT, op1=ADD)
                                scr = scrp.tile([C, 1024], FP8, name="scr")
                                nc.vector.tensor_scalar(
                                    out=scr[:], in0=P[:, row, csl],
                                    scalar1=1.0, scalar2=0.0,
                                    op0=MULT, op1=ADD, accum_out=zslot)
                            else:
                                nc.scalar.activation(
                                    out=P[:, row, csl], in_=st[:],
                                    func=EXP, scale=SCALE,
                                    bias=bias_t[:], accum_out=zslot)
                            if cch == 2:
                                drain_u(1)
                            if unit % wsb == wsb - 1:
                                drain(1)
                        # strip tail: Z, 1/Z, U unit
                        zs = zsp.tile([C, 1], F32, name="zs")
                        nc.vector.tensor_reduce(
                            out=zs[:], in_=zparts[:],
                            axis=mybir.AxisListType.X, op=ADD)
                        zr = zrp.tile([C, 1], F32, name="zr")
                        nc.vector.reciprocal(out=zr[:], in_=zs[:])
                        vq = sidx // 8
                        vcl = slice(128 * (sidx % 8), 128 * (sidx % 8 + 1))
                        up = pso.tile([C, C], F32, name="up", tag="po")
                        nc.tensor.matmul(
                            up[:], VT[32 * vq: 32 * (vq + 1), vcl],
                            wot[32 * vq: 32 * (vq + 1),
                                128 * h: 128 * (h + 1)],
                            start=True, stop=True,
                            tile_position=(32 * vq, 0))
                        pending_u.append(
                            lambda dst=Upairs[row // 2][:, row % 2, :],
                                   up=up, zr=zr:
                                nc.scalar.activation(
                                    out=dst, in_=up[:],
                                    func=mybir.ActivationFunctionType.Copy,
                                    scale=zr[:], bias=0.0))
                    drain_u(len(pending_u))
                    final = (h == 1 and w == len(wplan) - 1)
                    for ic in range(NIC):
                        pending.append(
                            lambda P=P, U=Upairs, ic=ic, fin=final:
                                emit_out_group(P, U, ic, fin))
                    wbase += wst
                cur_gv = next_gv
            drain(len(pending))

    nc.compile()
    return nc


def shard_inputs(x, Wp, bp, Wo):
    import ml_dtypes
    B, C_, H, W = x.shape
    xf = x.reshape(B, C_, H * W).astype(np.float32)
    in_maps = []
    for core in range(8):
        b = core // 2
        hp = core % 2
        heads = (2 * hp, 2 * hp + 1)
        wproj = np.empty((C_, 320), dtype=np.float32)
        ub = np.zeros((C_, 2), dtype=np.float32)
        vb = np.zeros((C_, 2), dtype=np.float32)
        wot = np.empty((32, 256), dtype=np.float32)
        for hi, h in enumerate(heads):
            Wq = Wp[96 * h: 96 * h + 32, :]          # (32, C)
            Wk = Wp[96 * h + 32: 96 * h + 64, :]
            Wv = Wp[96 * h + 64: 96 * h + 96, :]
            bq = bp[96 * h: 96 * h + 32]
            bk = bp[96 * h + 32: 96 * h + 64]
            bv = bp[96 * h + 64: 96 * h + 96]
            wproj[:, 160 * hi: 160 * hi + 128] = Wk.T @ Wq   # M^T
            wproj[:, 160 * hi + 128: 160 * hi + 160] = Wv.T
            ub[:, hi] = Wq.T @ bk       # u: survives along the i axis
            vb[:, hi] = np.tile(bv, 4)
            wo_h = Wo[:, 32 * h: 32 * (h + 1)] * USCALE
            wot[:, 128 * hi: 128 * (hi + 1)] = wo_h.T
        gamma = np.full((C_, 1), 1.0 if hp == 0 else 0.0, dtype=np.float32)
        in_maps.append({
            "xb": np.ascontiguousarray(xf[b]).astype(ml_dtypes.bfloat16),
            "wproj": wproj.astype(ml_dtypes.bfloat16),
            "wot": np.ascontiguousarray(
                np.tile(wot, (4, 1))).astype(ml_dtypes.bfloat16),
            "ub": ub,
            "vb": vb,
            "gamma": gamma,
        })
    return in_maps


def unshard_output(results, x_shape, bo):
    B, C_, H, W = x_shape
    out = np.empty((B, C_, H * W), dtype=np.float32)
    for b in range(B):
        out[b] = results[2 * b]["out"] + results[2 * b + 1]["out"] \
            + bo[:, None]
    return out.reshape(B, C_, H, W)


_NC_CACHE = []


def run(inputs, trace=False, tmpdir=None):
    """Run on 8 cores; returns (full_output, exec_time_ns_or_None)."""
    x = np.asarray(inputs["x"], dtype=np.float32)
    Wp = np.asarray(inputs["Wp"], dtype=np.float32)
    bp = np.asarray(inputs["bp"], dtype=np.float32)
    Wo = np.asarray(inputs["Wo"], dtype=np.float32)
    bo = np.asarray(inputs["bo"], dtype=np.float32)

    if not _NC_CACHE:
        _NC_CACHE.append(build_kernel())
    nc = _NC_CACHE[0]

    in_maps = shard_inputs(x, Wp, bp, Wo)
    kwargs = {}
    if trace:
        import tempfile
        kwargs = dict(trace=True,
                      tmpdir=tmpdir or tempfile.mkdtemp(prefix="attn_tr_"))
    res = bass_utils.run_bass_kernel_spmd(nc, in_maps,
                                          core_ids=list(range(8)), **kwargs)
    out = unshard_output(res.results, x.shape, bo)
    return out, res.exec_time_ns


def kernel(x, Wp, bp, Wo, bo):
    out, _ = run({"x": x, "Wp": Wp, "bp": bp, "Wo": Wo, "bo": bo})
    return out


# revision 18
# speedup vs baseline: 1.0134x; 1.0134x over previous
"""Trainium2 Bass kernel for nn_AttentionBlock_223338299515.

Reference (B=4, C=128, H=W=64, N=4096 tokens, 4 heads, d_k=32):
  xs = x.reshape(B,C,N).T ; qkv = xs @ Wp.T + bp ; q,k,v = split(qkv)
  attn = softmax_over_queries(q k^T / sqrt(dk)) ; res = attn-weighted v
  out = (res @ Wo.T + bo + xs).T -> (B, C, H, W)

8 NeuronCores, SPMD: core = (batch b = core//2, head pair hp = core%2).
All math in channel-transposed layout (x[b] used directly as (C, N)):
  S^T[j,i] = x_j^T (Wq^T Wk) x_i = sum_c G[c,j] x[c,i],  G = M^T x
    (M = Wq_h^T Wk_h is folded on the host, so Q/K never materialize:
     one G projection per head and x itself is the S-matmul moving side)
  P[j,i] = exp(S^T*scale - c)  (fp8e4; global shift c cancels in P/Z)
  Z[j]   = sum_i P[j,i]
  U[j,c] = (V^T.T @ WoT_h) * 4096/Z[j]   (fp8e4, merged attn-out+out-proj)
  out^T[c,i] = sum_{h,j} U[j,c] P[j,i] / 4096  (+ gamma*x residual)
Host: out[b] = core(2b) + core(2b+1) + bo.

qkv-bias folding: S^T gains (u.x_i) + (w.x_j + bq.bk) with u = Wk^T bq,
w = Wq^T bk.  The per-j part is constant along the softmax axis (i) and
cancels in P/Z, so only u survives — added per-partition during the
G-copy.  v-bias is added per-partition during the V-copy.  So one kernel
handles any bp.

The exp over the 33.5M-element S matrix is the bottleneck: ACT computes
most chunks natively to fp8 (accum_out gives Z for free); a share is
offloaded to DVE as a Schraudolph bit-trick exp (round(s*8/ln2 + B)
saturated to uint8, bitcast fp8e4) plus a DVE copy-with-accumulate for
those chunks' Z partials.  GPSIMD cannot touch PSUM, so it only does the
residual init.  Out-matmuls are fp8 DoubleRow (K=256, 0.5 cyc/col) over
strip pairs, accumulating 4 superblocks per PSUM tile before one DVE
read-modify-write into out_acc.
NOTE: HW fp8e4 encodes inf/NaN at exponent 15 (max normal 240, unlike
e4m3fn's 448) — the shift c keeps every fp8 value below 240.
"""
import os
import sys

import numpy as np

for _p in ("/opt/trn_rl_repo", "/root/.axon_site/_ro/trn_rl_repo"):
    if os.path.isdir(_p) and _p not in sys.path:
        sys.path.insert(0, _p)

import concourse.bacc as bacc
import concourse.tile as tile
from concourse import mybir
from concourse import bass_utils

F32 = mybir.dt.float32
F32R = mybir.dt.float32r
BF16 = mybir.dt.bfloat16
FP8 = mybir.dt.float8e4
U8 = mybir.dt.uint8
EXP = mybir.ActivationFunctionType.Exp
ADD = mybir.AluOpType.add
MULT = mybir.AluOpType.mult
DR = mybir.MatmulPerfMode.DoubleRow

N = 4096
C = 128
DK = 32
SCALE = float(DK) ** -0.5
NCH = 4            # 1024-col i-chunks per strip
NIC = 8            # 512-col out chunks
WSB = 4            # superblocks per window
NW = 8 // WSB      # windows per head
WST = 4 * WSB      # strips per window

CSHIFT = 1.77      # global logit shift; cancels in P/Z, keeps fp8 < 240
LN2 = float(np.log(2.0))
A8 = 8.0 / LN2
SC8 = SCALE * A8                              # Schraudolph scale on raw S
B8 = (7.0 - 0.0579) * 8.0 - CSHIFT * A8       # Schraudolph bias
USCALE = 4096.0

# exp engine schedule: True -> chunk offloaded to DVE Schraudolph (plus a
# DVE copy-with-accumulate for its Z partial). Bresenham over a 64-chunk
# period, interleaved so ACT and DVE consume the st ring concurrently.
NUM_O = 21
PAT_O = [((u * NUM_O) % 64) < NUM_O for u in range(64)]


def build_kernel():
    nc = bacc.Bacc("TRN2", target_bir_lowering=False, debug=False)

    xb_d = nc.dram_tensor("xb", (C, N), BF16, kind="ExternalInput")
    wproj_d = nc.dram_tensor("wproj", (C, 320), BF16, kind="ExternalInput")
    wot_d = nc.dram_tensor("wot", (C, 256), BF16, kind="ExternalInput")
    ub_d = nc.dram_tensor("ub", (C, 2), F32, kind="ExternalInput")
    vb_d = nc.dram_tensor("vb", (C, 2), F32, kind="ExternalInput")
    gamma_d = nc.dram_tensor("gamma", (C, 1), F32, kind="ExternalInput")
    out_d = nc.dram_tensor("out", (C, N), F32, kind="ExternalOutput")

    with tile.TileContext(nc) as tc:
        with (
            tc.tile_pool(name="const", bufs=1) as cpool,
            tc.tile_pool(name="gt", bufs=2) as gtp,
            tc.tile_pool(name="vt", bufs=2) as vtp,
            tc.tile_pool(name="pw", bufs=2) as ppool,
            tc.tile_pool(name="upair", bufs=16) as upool,
            tc.tile_pool(name="zp", bufs=6) as zpp,
            tc.tile_pool(name="zs", bufs=6) as zsp,
            tc.tile_pool(name="zr", bufs=8) as zrp,
            tc.tile_pool(name="scr", bufs=2) as scrp,
            tc.tile_pool(name="st", bufs=3, space="PSUM") as stp,
            tc.tile_pool(name="po", bufs=2, space="PSUM") as pso,
        ):
            xb = cpool.tile([C, N], BF16)
            for dc in range(8):
                dsl = slice(512 * dc, 512 * (dc + 1))
                nc.sync.dma_start(out=xb[:, dsl], in_=xb_d.ap()[:, dsl])
            wproj = cpool.tile([C, 320], BF16)
            nc.sync.dma_start(out=wproj[:], in_=wproj_d.ap())
            wot = cpool.tile([C, 256], BF16)
            nc.sync.dma_start(out=wot[:], in_=wot_d.ap())
            ub = cpool.tile([C, 2], F32)
            nc.sync.dma_start(out=ub[:], in_=ub_d.ap())
            vb = cpool.tile([C, 2], F32)
            nc.sync.dma_start(out=vb[:], in_=vb_d.ap())
            gamma = cpool.tile([C, 1], F32)
            nc.sync.dma_start(out=gamma[:], in_=gamma_d.ap())
            bias_t = cpool.tile([C, 1], F32)
            nc.vector.memset(bias_t[:], -CSHIFT)
            out_acc = cpool.tile([C, N], F32)

            # residual init: out_acc = gamma * x (gamma is 1 or 0)
            for dc in range(8):
                dsl = slice(512 * dc, 512 * (dc + 1))
                nc.vector.tensor_scalar(
                    out=out_acc[:, dsl], in0=xb[:, dsl],
                    scalar1=gamma[:], scalar2=None, op0=MULT)

            pending = []
            pending_u = []

            def drain(k=1):
                for _ in range(k):
                    if pending:
                        pending.pop(0)()

            def drain_u(k=1):
                for _ in range(k):
                    if pending_u:
                        pending_u.pop(0)()

            def alloc_gv(h):
                GT = gtp.tile([C, N], BF16, name=f"GT{h}", tag="GT")
                # V packed on partition quadrants: rows 32q..32q+31 hold
                # v-cols 1024q..1024q+1023 (wot is replicated to match).
                VT = vtp.tile([C, N // 4], BF16, name=f"VT{h}", tag="VT")
                return GT, VT

            def emit_gproj(h, gv, dc):
                GT, _ = gv
                pj = stp.tile([C, 1024], F32, name="pj", tag="st")
                csl = slice(1024 * dc, 1024 * (dc + 1))
                for half in range(2):
                    xsl = slice(1024 * dc + 512 * half,
                                1024 * dc + 512 * (half + 1))
                    nc.tensor.matmul(
                        pj[:, 512 * half: 512 * (half + 1)],
                        wproj[:, 160 * h: 160 * h + 128], xb[:, xsl],
                        start=True, stop=True)
                nc.scalar.activation(
                    out=GT[:, csl], in_=pj[:],
                    func=mybir.ActivationFunctionType.Identity,
                    scale=1.0, bias=ub[:, h: h + 1])

            def emit_vproj(h, gv):
                _, VT = gv
                pj = stp.tile([C, 1024], F32, name="pj", tag="st")
                for q in range(4):
                    for half in range(2):
                        xsl = slice(1024 * q + 512 * half,
                                    1024 * q + 512 * (half + 1))
                        nc.tensor.matmul(
                            pj[32 * q: 32 * (q + 1),
                               512 * half: 512 * (half + 1)],
                            wproj[:, 160 * h + 128: 160 * h + 160],
                            xb[:, xsl],
                            start=True, stop=True,
                            tile_position=(0, 32 * q))
                nc.scalar.activation(
                    out=VT[:], in_=pj[:],
                    func=mybir.ActivationFunctionType.Identity,
                    scale=1.0, bias=vb[:, h: h + 1])

            def emit_out_group(P, Upairs, ic, final):
                isl = slice(512 * ic, 512 * (ic + 1))
                op = pso.tile([C, 512], F32, name="op", tag="po")
                npair = len(Upairs)
                for t in range(npair):
                    nc.tensor.matmul(
                        op[:], Upairs[t][:], P[:, 2 * t: 2 * t + 2, isl],
                        start=(t == 0), stop=(t == npair - 1), perf_mode=DR)
                nc.vector.scalar_tensor_tensor(
                    out=out_acc[:, isl], in0=op[:], scalar=1.0 / USCALE,
                    in1=out_acc[:, isl], op0=MULT, op1=ADD)
                if final:
                    nc.sync.dma_start(out=out_d.ap()[:, isl],
                                      in_=out_acc[:, isl])

            cur_gv = alloc_gv(0)
            next_gv = None
            emit_gproj(0, cur_gv, 0)
            emit_vproj(0, cur_gv)

            for h in range(2):
                GT, VT = cur_gv
                wplan = [4, 4] if h == 0 else [4, 2, 2]
                wbase = 0
                for w, wsb in enumerate(wplan):
                    wst = 4 * wsb
                    P = ppool.tile([C, wst, N], FP8, name=f"P{w % 2}",
                                   tag="P")
                    Upairs = [upool.tile([C, 2, C], FP8, name=f"U{t}",
                                         tag="U") for t in range(wst // 2)]
                    for row in range(wst):
                        sidx = wbase + row
                        jsl = slice(128 * sidx, 128 * (sidx + 1))
                        zparts = zpp.tile([C, NCH], F32, name="zparts")
                        for cch in range(NCH):
                            unit = sidx * NCH + cch
                            if unit in (26, 58, 90):
                                emit_gproj(h, cur_gv, (unit + 6) // 32)
                            elif h == 0 and unit == 104:
                                next_gv = alloc_gv(1)
                                emit_gproj(1, next_gv, 0)
                            elif h == 0 and unit == 116:
                                emit_vproj(1, next_gv)
                            csl = slice(1024 * cch, 1024 * (cch + 1))
                            st = stp.tile([C, 1024], F32, name="st", tag="st")
                            for half in range(2):
                                xsl = slice(1024 * cch + 512 * half,
                                            1024 * cch + 512 * (half + 1))
                                nc.tensor.matmul(
                                    st[:, 512 * half: 512 * (half + 1)],
                                    GT[:, jsl], xb[:, xsl],
                                    start=True, stop=True)
                            zslot = zparts[:, cch: cch + 1]
                            if PAT_O[unit % 64]:
                                nc.vector.tensor_scalar(
                                    out=P[:, row, csl].bitcast(U8),
                                    in0=st[:], scalar1=SC8, scalar2=B8,
                                    op0=MULT, op1=ADD)
                                scr = scrp.tile([C, 1024], FP8, name="scr")
                                nc.vector.tensor_scalar(
                                    out=scr[:], in0=P[:, row, csl],
                                    scalar1=1.0, scalar2=0.0,
                                    op0=MULT, op1=ADD, accum_out=zslot)
                            else:
                                nc.scalar.activation(
                                    out=P[:, row, csl], in_=st[:],
                                    func=EXP, scale=SCALE,
                                    bias=bias_t[:], accum_out=zslot)
                            if cch == 2:
                                drain_u(1)
                            if unit % (2 * wsb) == 2 * wsb - 1:
                                drain(1)
                        # strip tail: Z, 1/Z, U unit
                        zs = zsp.tile([C, 1], F32, name="zs")
                        nc.vector.tensor_reduce(
                            out=zs[:], in_=zparts[:],
                            axis=mybir.AxisListType.X, op=ADD)
                        zr = zrp.tile([C, 1], F32, name="zr")
                        nc.vector.reciprocal(out=zr[:], in_=zs[:])
                        vq = sidx // 8
                        vcl = slice(128 * (sidx % 8), 128 * (sidx % 8 + 1))
                        up = pso.tile([C, C], F32, name="up", tag="po")
                        nc.tensor.matmul(
                            up[:], VT[32 * vq: 32 * (vq + 1), vcl],
                            wot[32 * vq: 32 * (vq + 1),
                                128 * h: 128 * (h + 1)],
                            start=True, stop=True,
                            tile_position=(32 * vq, 0))
                        pending_u.append(
                            lambda dst=Upairs[row // 2][:, row % 2, :],
                                   up=up, zr=zr:
                                nc.scalar.activation(
                                    out=dst, in_=up[:],
                                    func=mybir.ActivationFunctionType.Copy,
                                    scale=zr[:], bias=0.0))
                    drain_u(len(pending_u))
                    final = (h == 1 and w == len(wplan) - 1)
                    for ic in range(NIC):
                        pending.append(
                            lambda P=P, U=Upairs, ic=ic, fin=final:
                                emit_out_group(P, U, ic, fin))
                    wbase += wst
                cur_gv = next_gv
            drain(len(pending))

    nc.compile()
    return nc


def shard_inputs(x, Wp, bp, Wo):
    import ml_dtypes
    B, C_, H, W = x.shape
    xf = x.reshape(B, C_, H * W).astype(np.float32)
    in_maps = []
    for core in range(8):
        b = core // 2
        hp = core % 2
        heads = (2 * hp, 2 * hp + 1)
        wproj = np.empty((C_, 320), dtype=np.float32)
        ub = np.zeros((C_, 2), dtype=np.float32)
        vb = np.zeros((C_, 2), dtype=np.float32)
        wot = np.empty((32, 256), dtype=np.float32)
        for hi, h in enumerate(heads):
            Wq = Wp[96 * h: 96 * h + 32, :]          # (32, C)
            Wk = Wp[96 * h + 32: 96 * h + 64, :]
            Wv = Wp[96 * h + 64: 96 * h + 96, :]
            bq = bp[96 * h: 96 * h + 32]
            bk = bp[96 * h + 32: 96 * h + 64]
            bv = bp[96 * h + 64: 96 * h + 96]
            wproj[:, 160 * hi: 160 * hi + 128] = Wk.T @ Wq   # M^T
            wproj[:, 160 * hi + 128: 160 * hi + 160] = Wv.T
            ub[:, hi] = Wq.T @ bk       # u: survives along the i axis
            vb[:, hi] = np.tile(bv, 4)
            wo_h = Wo[:, 32 * h: 32 * (h + 1)] * USCALE
            wot[:, 128 * hi: 128 * (hi + 1)] = wo_h.T
        gamma = np.full((C_, 1), 1.0 if hp == 0 else 0.0, dtype=np.float32)
        in_maps.append({
            "xb": np.ascontiguousarray(xf[b]).astype(ml_dtypes.bfloat16),
            "wproj": wproj.astype(ml_dtypes.bfloat16),
            "wot": np.ascontiguousarray(
                np.tile(wot, (4, 1))).astype(ml_dtypes.bfloat16),
            "ub": ub,
            "vb": vb,
            "gamma": gamma,
        })
    return in_maps


def unshard_output(results, x_shape, bo):
    B, C_, H, W = x_shape
    out = np.empty((B, C_, H * W), dtype=np.float32)
    for b in range(B):
        out[b] = results[2 * b]["out"] + results[2 * b + 1]["out"] \
            + bo[:, None]
    return out.reshape(B, C_, H, W)


_NC_CACHE = []


def run(inputs, trace=False, tmpdir=None):
    """Run on 8 cores; returns (full_output, exec_time_ns_or_None)."""
    x = np.asarray(inputs["x"], dtype=np.float32)
    Wp = np.asarray(inputs["Wp"], dtype=np.float32)
    bp = np.asarray(inputs["bp"], dtype=np.float32)
    Wo = np.asarray(inputs["Wo"], dtype=np.float32)
    bo = np.asarray(inputs["bo"], dtype=np.float32)

    if not _NC_CACHE:
        _NC_CACHE.append(build_kernel())
    nc = _NC_CACHE[0]

    in_maps = shard_inputs(x, Wp, bp, Wo)
    kwargs = {}
    if trace:
        import tempfile
        kwargs = dict(trace=True,
                      tmpdir=tmpdir or tempfile.mkdtemp(prefix="attn_tr_"))
    res = bass_utils.run_bass_kernel_spmd(nc, in_maps,
                                          core_ids=list(range(8)), **kwargs)
    out = unshard_output(res.results, x.shape, bo)
    return out, res.exec_time_ns


def kernel(x, Wp, bp, Wo, bo):
    out, _ = run({"x": x, "Wp": Wp, "bp": bp, "Wo": Wo, "bo": bo})
    return out


# revision 19
# speedup vs baseline: 1.0300x; 1.0164x over previous
"""Trainium2 Bass kernel for nn_AttentionBlock_223338299515.

Reference (B=4, C=128, H=W=64, N=4096 tokens, 4 heads, d_k=32):
  xs = x.reshape(B,C,N).T ; qkv = xs @ Wp.T + bp ; q,k,v = split(qkv)
  attn = softmax_over_queries(q k^T / sqrt(dk)) ; res = attn-weighted v
  out = (res @ Wo.T + bo + xs).T -> (B, C, H, W)

8 NeuronCores, SPMD: core = (batch b = core//2, head pair hp = core%2).
All math in channel-transposed layout (x[b] used directly as (C, N)):
  S^T[j,i] = x_j^T (Wq^T Wk) x_i = sum_c G[c,j] x[c,i],  G = M^T x
    (M = Wq_h^T Wk_h is folded on the host, so Q/K never materialize:
     one G projection per head and x itself is the S-matmul moving side)
  P[j,i] = exp(S^T*scale - c)  (fp8e4; global shift c cancels in P/Z)
  Z[j]   = sum_i P[j,i]
  U[j,c] = (V^T.T @ WoT_h) * 4096/Z[j]   (fp8e4, merged attn-out+out-proj)
  out^T[c,i] = sum_{h,j} U[j,c] P[j,i] / 4096  (+ gamma*x residual)
Host: out[b] = core(2b) + core(2b+1) + bo.

qkv-bias folding: S^T gains (u.x_i) + (w.x_j + bq.bk) with u = Wk^T bq,
w = Wq^T bk.  The per-j part is constant along the softmax axis (i) and
cancels in P/Z, so only u survives — added per-partition during the
G-copy.  v-bias is added per-partition during the V-copy.  So one kernel
handles any bp.

The exp over the 33.5M-element S matrix is the bottleneck: ACT computes
most chunks natively to fp8 (accum_out gives Z for free); a share is
offloaded to DVE as a Schraudolph bit-trick exp (round(s*8/ln2 + B)
saturated to uint8, bitcast fp8e4) plus a DVE copy-with-accumulate for
those chunks' Z partials.  GPSIMD cannot touch PSUM, so it only does the
residual init.  Out-matmuls are fp8 DoubleRow (K=256, 0.5 cyc/col) over
strip pairs, accumulating 4 superblocks per PSUM tile before one DVE
read-modify-write into out_acc.
NOTE: HW fp8e4 encodes inf/NaN at exponent 15 (max normal 240, unlike
e4m3fn's 448) — the shift c keeps every fp8 value below 240.
"""
import os
import sys

import numpy as np

for _p in ("/opt/trn_rl_repo", "/root/.axon_site/_ro/trn_rl_repo"):
    if os.path.isdir(_p) and _p not in sys.path:
        sys.path.insert(0, _p)

import concourse.bacc as bacc
import concourse.tile as tile
from concourse import mybir
from concourse import bass_utils

F32 = mybir.dt.float32
F32R = mybir.dt.float32r
BF16 = mybir.dt.bfloat16
FP8 = mybir.dt.float8e4
U8 = mybir.dt.uint8
EXP = mybir.ActivationFunctionType.Exp
ADD = mybir.AluOpType.add
MULT = mybir.AluOpType.mult
DR = mybir.MatmulPerfMode.DoubleRow

N = 4096
C = 128
DK = 32
SCALE = float(DK) ** -0.5
NCH = 4            # 1024-col i-chunks per strip
NIC = 8            # 512-col out chunks
WSB = 4            # superblocks per window
NW = 8 // WSB      # windows per head
WST = 4 * WSB      # strips per window

CSHIFT = 1.77      # global logit shift; cancels in P/Z, keeps fp8 < 240
LN2 = float(np.log(2.0))
A8 = 8.0 / LN2
SC8 = SCALE * A8                              # Schraudolph scale on raw S
B8 = (7.0 - 0.0579) * 8.0 - CSHIFT * A8       # Schraudolph bias
USCALE = 4096.0

# exp engine schedule: True -> chunk offloaded to DVE Schraudolph (plus a
# DVE copy-with-accumulate for its Z partial). Bresenham over a 64-chunk
# period, interleaved so ACT and DVE consume the st ring concurrently.
NUM_O = 21
PAT_O = [((u * NUM_O) % 64) < NUM_O for u in range(64)]


def build_kernel():
    nc = bacc.Bacc("TRN2", target_bir_lowering=False, debug=False)

    xb_d = nc.dram_tensor("xb", (C, N), BF16, kind="ExternalInput")
    wproj_d = nc.dram_tensor("wproj", (C, 320), BF16, kind="ExternalInput")
    wot_d = nc.dram_tensor("wot", (C, 256), BF16, kind="ExternalInput")
    ub_d = nc.dram_tensor("ub", (C, 2), F32, kind="ExternalInput")
    vb_d = nc.dram_tensor("vb", (C, 2), F32, kind="ExternalInput")
    gamma_d = nc.dram_tensor("gamma", (C, 1), F32, kind="ExternalInput")
    out_d = nc.dram_tensor("out", (C, N), F32, kind="ExternalOutput")

    with tile.TileContext(nc) as tc:
        with (
            tc.tile_pool(name="const", bufs=1) as cpool,
            tc.tile_pool(name="gt", bufs=2) as gtp,
            tc.tile_pool(name="vt", bufs=2) as vtp,
            tc.tile_pool(name="pw", bufs=2) as ppool,
            tc.tile_pool(name="upair", bufs=16) as upool,
            tc.tile_pool(name="zp", bufs=6) as zpp,
            tc.tile_pool(name="zs", bufs=6) as zsp,
            tc.tile_pool(name="zr", bufs=8) as zrp,
            tc.tile_pool(name="scr", bufs=2) as scrp,
            tc.tile_pool(name="st", bufs=3, space="PSUM") as stp,
            tc.tile_pool(name="po", bufs=2, space="PSUM") as pso,
        ):
            xb = cpool.tile([C, N], BF16)
            for dc in range(4):
                dsl = slice(1024 * dc, 1024 * (dc + 1))
                nc.sync.dma_start(out=xb[:, dsl], in_=xb_d.ap()[:, dsl])
            wproj = cpool.tile([C, 320], BF16)
            nc.sync.dma_start(out=wproj[:], in_=wproj_d.ap())
            wot = cpool.tile([C, 256], BF16)
            nc.sync.dma_start(out=wot[:], in_=wot_d.ap())
            ub = cpool.tile([C, 2], F32)
            nc.sync.dma_start(out=ub[:], in_=ub_d.ap())
            vb = cpool.tile([C, 2], F32)
            nc.sync.dma_start(out=vb[:], in_=vb_d.ap())
            gamma = cpool.tile([C, 1], F32)
            nc.sync.dma_start(out=gamma[:], in_=gamma_d.ap())
            bias_t = cpool.tile([C, 1], F32)
            nc.vector.memset(bias_t[:], -CSHIFT)
            out_acc = cpool.tile([C, N], F32)

            # residual init: out_acc = gamma * x (gamma is 1 or 0)
            for dc in range(8):
                dsl = slice(512 * dc, 512 * (dc + 1))
                nc.vector.tensor_scalar(
                    out=out_acc[:, dsl], in0=xb[:, dsl],
                    scalar1=gamma[:], scalar2=None, op0=MULT)

            pending = []
            pending_u = []

            def drain(k=1):
                for _ in range(k):
                    if pending:
                        pending.pop(0)()

            def drain_u(k=1):
                for _ in range(k):
                    if pending_u:
                        pending_u.pop(0)()

            def alloc_gv(h):
                GT = gtp.tile([C, N], BF16, name=f"GT{h}", tag="GT")
                # V packed on partition quadrants: rows 32q..32q+31 hold
                # v-cols 1024q..1024q+1023 (wot is replicated to match).
                VT = vtp.tile([C, N // 4], BF16, name=f"VT{h}", tag="VT")
                return GT, VT

            def emit_gproj(h, gv, dc):
                GT, _ = gv
                pj = stp.tile([C, 1024], F32, name="pj", tag="st")
                csl = slice(1024 * dc, 1024 * (dc + 1))
                for half in range(2):
                    xsl = slice(1024 * dc + 512 * half,
                                1024 * dc + 512 * (half + 1))
                    nc.tensor.matmul(
                        pj[:, 512 * half: 512 * (half + 1)],
                        wproj[:, 160 * h: 160 * h + 128], xb[:, xsl],
                        start=True, stop=True)
                nc.scalar.activation(
                    out=GT[:, csl], in_=pj[:],
                    func=mybir.ActivationFunctionType.Identity,
                    scale=1.0, bias=ub[:, h: h + 1])

            def emit_vproj(h, gv):
                _, VT = gv
                pj = stp.tile([C, 1024], F32, name="pj", tag="st")
                for q in range(4):
                    for half in range(2):
                        xsl = slice(1024 * q + 512 * half,
                                    1024 * q + 512 * (half + 1))
                        nc.tensor.matmul(
                            pj[32 * q: 32 * (q + 1),
                               512 * half: 512 * (half + 1)],
                            wproj[:, 160 * h + 128: 160 * h + 160],
                            xb[:, xsl],
                            start=True, stop=True,
                            tile_position=(0, 32 * q))
                nc.scalar.activation(
                    out=VT[:], in_=pj[:],
                    func=mybir.ActivationFunctionType.Identity,
                    scale=1.0, bias=vb[:, h: h + 1])

            def emit_out_group(P, Upairs, ic, final):
                isl = slice(512 * ic, 512 * (ic + 1))
                op = pso.tile([C, 512], F32, name="op", tag="po")
                npair = len(Upairs)
                for t in range(npair):
                    nc.tensor.matmul(
                        op[:], Upairs[t][:], P[:, 2 * t: 2 * t + 2, isl],
                        start=(t == 0), stop=(t == npair - 1), perf_mode=DR)
                nc.vector.scalar_tensor_tensor(
                    out=out_acc[:, isl], in0=op[:], scalar=1.0 / USCALE,
                    in1=out_acc[:, isl], op0=MULT, op1=ADD)
                if final:
                    nc.sync.dma_start(out=out_d.ap()[:, isl],
                                      in_=out_acc[:, isl])

            cur_gv = alloc_gv(0)
            next_gv = None
            emit_gproj(0, cur_gv, 0)
            emit_vproj(0, cur_gv)

            for h in range(2):
                GT, VT = cur_gv
                wplan = [4, 4] if h == 0 else [4, 2, 2]
                wbase = 0
                for w, wsb in enumerate(wplan):
                    wst = 4 * wsb
                    P = ppool.tile([C, wst, N], FP8, name=f"P{w % 2}",
                                   tag="P")
                    Upairs = [upool.tile([C, 2, C], FP8, name=f"U{t}",
                                         tag="U") for t in range(wst // 2)]
                    for row in range(wst):
                        sidx = wbase + row
                        jsl = slice(128 * sidx, 128 * (sidx + 1))
                        zparts = zpp.tile([C, NCH], F32, name="zparts")
                        for cch in range(NCH):
                            unit = sidx * NCH + cch
                            if unit in (26, 58, 90):
                                emit_gproj(h, cur_gv, (unit + 6) // 32)
                            elif h == 0 and unit == 104:
                                next_gv = alloc_gv(1)
                                emit_gproj(1, next_gv, 0)
                            elif h == 0 and unit == 116:
                                emit_vproj(1, next_gv)
                            csl = slice(1024 * cch, 1024 * (cch + 1))
                            st = stp.tile([C, 1024], F32, name="st", tag="st")
                            for half in range(2):
                                xsl = slice(1024 * cch + 512 * half,
                                            1024 * cch + 512 * (half + 1))
                                nc.tensor.matmul(
                                    st[:, 512 * half: 512 * (half + 1)],
                                    GT[:, jsl], xb[:, xsl],
                                    start=True, stop=True)
                            zslot = zparts[:, cch: cch + 1]
                            if PAT_O[unit % 64]:
                                nc.vector.tensor_scalar(
                                    out=P[:, row, csl].bitcast(U8),
                                    in0=st[:], scalar1=SC8, scalar2=B8,
                                    op0=MULT, op1=ADD)
                                scr = scrp.tile([C, 1024], FP8, name="scr")
                                nc.vector.tensor_scalar(
                                    out=scr[:], in0=P[:, row, csl],
                                    scalar1=1.0, scalar2=0.0,
                                    op0=MULT, op1=ADD, accum_out=zslot)
                            else:
                                nc.scalar.activation(
                                    out=P[:, row, csl], in_=st[:],
                                    func=EXP, scale=SCALE,
                                    bias=bias_t[:], accum_out=zslot)
                            if cch == 2:
                                drain_u(1)
                            if unit % (2 * wsb) == 2 * wsb - 1:
                                drain(1)
                        # strip tail: Z, 1/Z, U unit
                        zs = zsp.tile([C, 1], F32, name="zs")
                        nc.vector.tensor_reduce(
                            out=zs[:], in_=zparts[:],
                            axis=mybir.AxisListType.X, op=ADD)
                        zr = zrp.tile([C, 1], F32, name="zr")
                        nc.vector.reciprocal(out=zr[:], in_=zs[:])
                        vq = sidx // 8
                        vcl = slice(128 * (sidx % 8), 128 * (sidx % 8 + 1))
                        up = pso.tile([C, C], F32, name="up", tag="po")
                        nc.tensor.matmul(
                            up[:], VT[32 * vq: 32 * (vq + 1), vcl],
                            wot[32 * vq: 32 * (vq + 1),
                                128 * h: 128 * (h + 1)],
                            start=True, stop=True,
                            tile_position=(32 * vq, 0))
                        pending_u.append(
                            lambda dst=Upairs[row // 2][:, row % 2, :],
                                   up=up, zr=zr:
                                nc.scalar.activation(
                                    out=dst, in_=up[:],
                                    func=mybir.ActivationFunctionType.Copy,
                                    scale=zr[:], bias=0.0))
                    drain_u(len(pending_u))
                    final = (h == 1 and w == len(wplan) - 1)
                    for ic in range(NIC):
                        pending.append(
                            lambda P=P, U=Upairs, ic=ic, fin=final:
                                emit_out_group(P, U, ic, fin))
                    wbase += wst
                cur_gv = next_gv
            drain(len(pending))

    nc.compile()
    return nc


def shard_inputs(x, Wp, bp, Wo):
    import ml_dtypes
    B, C_, H, W = x.shape
    xf = x.reshape(B, C_, H * W).astype(np.float32)
    in_maps = []
    for core in range(8):
        b = core // 2
        hp = core % 2
        heads = (2 * hp, 2 * hp + 1)
        wproj = np.empty((C_, 320), dtype=np.float32)
        ub = np.zeros((C_, 2), dtype=np.float32)
        vb = np.zeros((C_, 2), dtype=np.float32)
        wot = np.empty((32, 256), dtype=np.float32)
        for hi, h in enumerate(heads):
            Wq = Wp[96 * h: 96 * h + 32, :]          # (32, C)
            Wk = Wp[96 * h + 32: 96 * h + 64, :]
            Wv = Wp[96 * h + 64: 96 * h + 96, :]
            bq = bp[96 * h: 96 * h + 32]
            bk = bp[96 * h + 32: 96 * h + 64]
            bv = bp[96 * h + 64: 96 * h + 96]
            wproj[:, 160 * hi: 160 * hi + 128] = Wk.T @ Wq   # M^T
            wproj[:, 160 * hi + 128: 160 * hi + 160] = Wv.T
            ub[:, hi] = Wq.T @ bk       # u: survives along the i axis
            vb[:, hi] = np.tile(bv, 4)
            wo_h = Wo[:, 32 * h: 32 * (h + 1)] * USCALE
            wot[:, 128 * hi: 128 * (hi + 1)] = wo_h.T
        gamma = np.full((C_, 1), 1.0 if hp == 0 else 0.0, dtype=np.float32)
        in_maps.append({
            "xb": np.ascontiguousarray(xf[b]).astype(ml_dtypes.bfloat16),
            "wproj": wproj.astype(ml_dtypes.bfloat16),
            "wot": np.ascontiguousarray(
                np.tile(wot, (4, 1))).astype(ml_dtypes.bfloat16),
            "ub": ub,
            "vb": vb,
            "gamma": gamma,
        })
    return in_maps


def unshard_output(results, x_shape, bo):
    B, C_, H, W = x_shape
    out = np.empty((B, C_, H * W), dtype=np.float32)
    for b in range(B):
        out[b] = results[2 * b]["out"] + results[2 * b + 1]["out"] \
            + bo[:, None]
    return out.reshape(B, C_, H, W)


_NC_CACHE = []


def run(inputs, trace=False, tmpdir=None):
    """Run on 8 cores; returns (full_output, exec_time_ns_or_None)."""
    x = np.asarray(inputs["x"], dtype=np.float32)
    Wp = np.asarray(inputs["Wp"], dtype=np.float32)
    bp = np.asarray(inputs["bp"], dtype=np.float32)
    Wo = np.asarray(inputs["Wo"], dtype=np.float32)
    bo = np.asarray(inputs["bo"], dtype=np.float32)

    if not _NC_CACHE:
        _NC_CACHE.append(build_kernel())
    nc = _NC_CACHE[0]

    in_maps = shard_inputs(x, Wp, bp, Wo)
    kwargs = {}
    if trace:
        import tempfile
        kwargs = dict(trace=True,
                      tmpdir=tmpdir or tempfile.mkdtemp(prefix="attn_tr_"))
    res = bass_utils.run_bass_kernel_spmd(nc, in_maps,
                                          core_ids=list(range(8)), **kwargs)
    out = unshard_output(res.results, x.shape, bo)
    return out, res.exec_time_ns


def kernel(x, Wp, bp, Wo, bo):
    out, _ = run({"x": x, "Wp": Wp, "bp": bp, "Wo": Wo, "bo": bo})
    return out


# revision 20
# speedup vs baseline: 1.2256x; 1.1899x over previous
"""Trainium2 Bass kernel for nn_AttentionBlock_223338299515.

Reference (B=4, C=128, H=W=64, N=4096 tokens, 4 heads, d_k=32):
  xs = x.reshape(B,C,N).T ; qkv = xs @ Wp.T + bp ; q,k,v = split(qkv)
  attn = softmax_over_queries(q k^T / sqrt(dk)) ; res = attn-weighted v
  out = (res @ Wo.T + bo + xs).T -> (B, C, H, W)

8 NeuronCores, SPMD: core = (batch b = core//2, head pair hp = core%2).
All math in channel-transposed layout (x[b] used directly as (C, N)):
  S^T[j,i] = x_j^T (Wq^T Wk) x_i = sum_c G[c,j] x[c,i],  G = M^T x
    (M = Wq_h^T Wk_h is folded on the host, so Q/K never materialize:
     one G projection per head and x itself is the S-matmul moving side)
  P[j,i] = exp(S^T*scale - c)  (fp8e4; global shift c cancels in P/Z)
  Z[j]   = sum_i P[j,i]
  U[j,c] = (V^T.T @ WoT_h) * 4096/Z[j]   (fp8e4, merged attn-out+out-proj)
  out^T[c,i] = sum_{h,j} U[j,c] P[j,i] / 4096  (+ gamma*x residual)
Host: out[b] = core(2b) + core(2b+1) + bo.

qkv-bias folding: S^T gains (u.x_i) + (w.x_j + bq.bk) with u = Wk^T bq,
w = Wq^T bk.  The per-j part is constant along the softmax axis (i) and
cancels in P/Z, so only u survives — added per-partition during the
G-copy.  v-bias is added per-partition during the V-copy.  So one kernel
handles any bp.

The exp over the 33.5M-element S matrix is the bottleneck: ACT computes
most chunks natively to fp8 (accum_out gives Z for free); a share is
offloaded to DVE as a Schraudolph bit-trick exp (round(s*8/ln2 + B)
saturated to uint8, bitcast fp8e4) plus a DVE copy-with-accumulate for
those chunks' Z partials.  GPSIMD cannot touch PSUM, so it only does the
residual init.  Out-matmuls are fp8 DoubleRow (K=256, 0.5 cyc/col) over
strip pairs, accumulating 4 superblocks per PSUM tile before one DVE
read-modify-write into out_acc.
NOTE: HW fp8e4 encodes inf/NaN at exponent 15 (max normal 240, unlike
e4m3fn's 448) — the shift c keeps every fp8 value below 240.
"""
import os
import sys

import numpy as np

for _p in ("/opt/trn_rl_repo", "/root/.axon_site/_ro/trn_rl_repo"):
    if os.path.isdir(_p) and _p not in sys.path:
        sys.path.insert(0, _p)

import concourse.bacc as bacc
import concourse.tile as tile
from concourse import mybir
from concourse import bass_utils

F32 = mybir.dt.float32
F32R = mybir.dt.float32r
BF16 = mybir.dt.bfloat16
FP8 = mybir.dt.float8e4
U8 = mybir.dt.uint8
EXP = mybir.ActivationFunctionType.Exp
ADD = mybir.AluOpType.add
MULT = mybir.AluOpType.mult
DR = mybir.MatmulPerfMode.DoubleRow

N = 4096
C = 128
DK = 32
SCALE = float(DK) ** -0.5
NCH = 4            # 1024-col i-chunks per strip
NIC = 8            # 512-col out chunks
WSB = 4            # superblocks per window
NW = 8 // WSB      # windows per head
WST = 4 * WSB      # strips per window

CSHIFT = 1.77      # global logit shift; cancels in P/Z, keeps fp8 < 240
LN2 = float(np.log(2.0))
A8 = 8.0 / LN2
SC8 = SCALE * A8                              # Schraudolph scale on raw S
B8 = (7.0 - 0.0579) * 8.0 - CSHIFT * A8       # Schraudolph bias
USCALE = 4096.0

# exp engine schedule: True -> chunk offloaded to DVE Schraudolph (plus a
# DVE copy-with-accumulate for its Z partial). Bresenham over a 64-chunk
# period, interleaved so ACT and DVE consume the st ring concurrently.
NUM_O = 21
PAT_O = [((u * NUM_O) % 64) < NUM_O for u in range(64)]


def build_kernel():
    nc = bacc.Bacc("TRN2", target_bir_lowering=False, debug=False)

    xb_d = nc.dram_tensor("xb", (C, N), BF16, kind="ExternalInput")
    wproj_d = nc.dram_tensor("wproj", (C, 320), BF16, kind="ExternalInput")
    wot_d = nc.dram_tensor("wot", (C, 256), BF16, kind="ExternalInput")
    ub_d = nc.dram_tensor("ub", (C, 2), F32, kind="ExternalInput")
    vb_d = nc.dram_tensor("vb", (C, 2), F32, kind="ExternalInput")
    gamma_d = nc.dram_tensor("gamma", (C, 1), F32, kind="ExternalInput")
    out_d = nc.dram_tensor("out", (C, N), F32, kind="ExternalOutput")

    with tile.TileContext(nc) as tc:
        with (
            tc.tile_pool(name="const", bufs=1) as cpool,
            tc.tile_pool(name="gt", bufs=2) as gtp,
            tc.tile_pool(name="vt", bufs=2) as vtp,
            tc.tile_pool(name="pw", bufs=2) as ppool,
            tc.tile_pool(name="upair", bufs=16) as upool,
            tc.tile_pool(name="zp", bufs=6) as zpp,
            tc.tile_pool(name="zs", bufs=6) as zsp,
            tc.tile_pool(name="zr", bufs=8) as zrp,
            tc.tile_pool(name="scr", bufs=2) as scrp,
            tc.tile_pool(name="st", bufs=3, space="PSUM") as stp,
            tc.tile_pool(name="po", bufs=2, space="PSUM") as pso,
        ):
            wproj = cpool.tile([C, 320], BF16)
            nc.sync.dma_start(out=wproj[:], in_=wproj_d.ap())
            ub = cpool.tile([C, 2], F32)
            nc.sync.dma_start(out=ub[:], in_=ub_d.ap())
            xb = cpool.tile([C, N], BF16)
            for dc in range(4):
                dsl = slice(1024 * dc, 1024 * (dc + 1))
                nc.sync.dma_start(out=xb[:, dsl], in_=xb_d.ap()[:, dsl])
            wot = cpool.tile([C, 256], BF16)
            nc.sync.dma_start(out=wot[:], in_=wot_d.ap())
            vb = cpool.tile([C, 2], F32)
            nc.sync.dma_start(out=vb[:], in_=vb_d.ap())
            gamma = cpool.tile([C, 1], F32)
            nc.sync.dma_start(out=gamma[:], in_=gamma_d.ap())
            bias_t = cpool.tile([C, 1], F32)
            nc.vector.memset(bias_t[:], -CSHIFT)
            out_acc = cpool.tile([C, N], F32)

            pending = []
            pending_u = []

            def drain(k=1):
                for _ in range(k):
                    if pending:
                        pending.pop(0)()

            def drain_u(k=1):
                for _ in range(k):
                    if pending_u:
                        pending_u.pop(0)()

            def alloc_gv(h):
                GT = gtp.tile([C, N], BF16, name=f"GT{h}", tag="GT")
                # V packed on partition quadrants: rows 32q..32q+31 hold
                # v-cols 1024q..1024q+1023 (wot is replicated to match).
                VT = vtp.tile([C, N // 4], BF16, name=f"VT{h}", tag="VT")
                return GT, VT

            def emit_gproj(h, gv, dc):
                GT, _ = gv
                pj = stp.tile([C, 1024], F32, name="pj", tag="st")
                csl = slice(1024 * dc, 1024 * (dc + 1))
                for half in range(2):
                    xsl = slice(1024 * dc + 512 * half,
                                1024 * dc + 512 * (half + 1))
                    nc.tensor.matmul(
                        pj[:, 512 * half: 512 * (half + 1)],
                        wproj[:, 160 * h: 160 * h + 128], xb[:, xsl],
                        start=True, stop=True)
                nc.scalar.activation(
                    out=GT[:, csl], in_=pj[:],
                    func=mybir.ActivationFunctionType.Identity,
                    scale=1.0, bias=ub[:, h: h + 1])

            def emit_vproj(h, gv):
                _, VT = gv
                pj = stp.tile([C, 1024], F32, name="pj", tag="st")
                for q in range(4):
                    for half in range(2):
                        xsl = slice(1024 * q + 512 * half,
                                    1024 * q + 512 * (half + 1))
                        nc.tensor.matmul(
                            pj[32 * q: 32 * (q + 1),
                               512 * half: 512 * (half + 1)],
                            wproj[:, 160 * h + 128: 160 * h + 160],
                            xb[:, xsl],
                            start=True, stop=True,
                            tile_position=(0, 32 * q))
                nc.scalar.activation(
                    out=VT[:], in_=pj[:],
                    func=mybir.ActivationFunctionType.Identity,
                    scale=1.0, bias=vb[:, h: h + 1])

            def emit_out_group(P, Upairs, ic, final):
                isl = slice(512 * ic, 512 * (ic + 1))
                op = pso.tile([C, 512], F32, name="op", tag="po")
                npair = len(Upairs)
                for t in range(npair):
                    nc.tensor.matmul(
                        op[:], Upairs[t][:], P[:, 2 * t: 2 * t + 2, isl],
                        start=(t == 0), stop=(t == npair - 1), perf_mode=DR)
                nc.vector.scalar_tensor_tensor(
                    out=out_acc[:, isl], in0=op[:], scalar=1.0 / USCALE,
                    in1=out_acc[:, isl], op0=MULT, op1=ADD)
                if final:
                    nc.sync.dma_start(out=out_d.ap()[:, isl],
                                      in_=out_acc[:, isl])

            cur_gv = alloc_gv(0)
            next_gv = None
            emit_gproj(0, cur_gv, 0)
            emit_vproj(0, cur_gv)

            for h in range(2):
                GT, VT = cur_gv
                wplan = [4, 4] if h == 0 else [4, 2, 2]
                wbase = 0
                for w, wsb in enumerate(wplan):
                    wst = 4 * wsb
                    P = ppool.tile([C, wst, N], FP8, name=f"P{w % 2}",
                                   tag="P")
                    Upairs = [upool.tile([C, 2, C], FP8, name=f"U{t}",
                                         tag="U") for t in range(wst // 2)]
                    for row in range(wst):
                        sidx = wbase + row
                        jsl = slice(128 * sidx, 128 * (sidx + 1))
                        zparts = zpp.tile([C, NCH], F32, name="zparts")
                        for cch in range(NCH):
                            unit = sidx * NCH + cch
                            if h == 0 and w == 0 and unit == 12:
                                for gdc in range(8):
                                    gsl = slice(512 * gdc, 512 * (gdc + 1))
                                    nc.vector.tensor_scalar(
                                        out=out_acc[:, gsl],
                                        in0=xb[:, gsl],
                                        scalar1=gamma[:], scalar2=None,
                                        op0=MULT)
                            if unit in (26, 58, 90):
                                emit_gproj(h, cur_gv, (unit + 6) // 32)
                            elif h == 0 and unit == 104:
                                next_gv = alloc_gv(1)
                                emit_gproj(1, next_gv, 0)
                            elif h == 0 and unit == 116:
                                emit_vproj(1, next_gv)
                            csl = slice(1024 * cch, 1024 * (cch + 1))
                            st = stp.tile([C, 1024], F32, name="st", tag="st")
                            for half in range(2):
                                xsl = slice(1024 * cch + 512 * half,
                                            1024 * cch + 512 * (half + 1))
                                nc.tensor.matmul(
                                    st[:, 512 * half: 512 * (half + 1)],
                                    GT[:, jsl], xb[:, xsl],
                                    start=True, stop=True)
                            zslot = zparts[:, cch: cch + 1]
                            if PAT_O[unit % 64]:
                                nc.vector.tensor_scalar(
                                    out=P[:, row, csl].bitcast(U8),
                                    in0=st[:], scalar1=SC8, scalar2=B8,
                                    op0=MULT, op1=ADD)
                                scr = scrp.tile([C, 1024], FP8, name="scr")
                                nc.vector.tensor_scalar(
                                    out=scr[:], in0=P[:, row, csl],
                                    scalar1=1.0, scalar2=0.0,
                                    op0=MULT, op1=ADD, accum_out=zslot)
                            else:
                                nc.scalar.activation(
                                    out=P[:, row, csl], in_=st[:],
                                    func=EXP, scale=SCALE,
                                    bias=bias_t[:], accum_out=zslot)
                            if cch == 2:
                                drain_u(1)
                            if unit % (2 * wsb) == 2 * wsb - 1:
                                drain(1)
                        # strip tail: Z, 1/Z, U unit
                        zs = zsp.tile([C, 1], F32, name="zs")
                        nc.vector.tensor_reduce(
                            out=zs[:], in_=zparts[:],
                            axis=mybir.AxisListType.X, op=ADD)
                        zr = zrp.tile([C, 1], F32, name="zr")
                        nc.vector.reciprocal(out=zr[:], in_=zs[:])
                        vq = sidx // 8
                        vcl = slice(128 * (sidx % 8), 128 * (sidx % 8 + 1))
                        up = pso.tile([C, C], F32, name="up", tag="po")
                        nc.tensor.matmul(
                            up[:], VT[32 * vq: 32 * (vq + 1), vcl],
                            wot[32 * vq: 32 * (vq + 1),
                                128 * h: 128 * (h + 1)],
                            start=True, stop=True,
                            tile_position=(32 * vq, 0))
                        pending_u.append(
                            lambda dst=Upairs[row // 2][:, row % 2, :],
                                   up=up, zr=zr:
                                nc.scalar.activation(
                                    out=dst, in_=up[:],
                                    func=mybir.ActivationFunctionType.Copy,
                                    scale=zr[:], bias=0.0))
                    drain_u(len(pending_u))
                    final = (h == 1 and w == len(wplan) - 1)
                    for ic in range(NIC):
                        pending.append(
                            lambda P=P, U=Upairs, ic=ic, fin=final:
                                emit_out_group(P, U, ic, fin))
                    wbase += wst
                cur_gv = next_gv
            drain(len(pending))

    nc.compile()
    return nc


def shard_inputs(x, Wp, bp, Wo):
    import ml_dtypes
    B, C_, H, W = x.shape
    xf = x.reshape(B, C_, H * W).astype(np.float32)
    in_maps = []
    for core in range(8):
        b = core // 2
        hp = core % 2
        heads = (2 * hp, 2 * hp + 1)
        wproj = np.empty((C_, 320), dtype=np.float32)
        ub = np.zeros((C_, 2), dtype=np.float32)
        vb = np.zeros((C_, 2), dtype=np.float32)
        wot = np.empty((32, 256), dtype=np.float32)
        for hi, h in enumerate(heads):
            Wq = Wp[96 * h: 96 * h + 32, :]          # (32, C)
            Wk = Wp[96 * h + 32: 96 * h + 64, :]
            Wv = Wp[96 * h + 64: 96 * h + 96, :]
            bq = bp[96 * h: 96 * h + 32]
            bk = bp[96 * h + 32: 96 * h + 64]
            bv = bp[96 * h + 64: 96 * h + 96]
            wproj[:, 160 * hi: 160 * hi + 128] = Wk.T @ Wq   # M^T
            wproj[:, 160 * hi + 128: 160 * hi + 160] = Wv.T
            ub[:, hi] = Wq.T @ bk       # u: survives along the i axis
            vb[:, hi] = np.tile(bv, 4)
            wo_h = Wo[:, 32 * h: 32 * (h + 1)] * USCALE
            wot[:, 128 * hi: 128 * (hi + 1)] = wo_h.T
        gamma = np.full((C_, 1), 1.0 if hp == 0 else 0.0, dtype=np.float32)
        in_maps.append({
            "xb": np.ascontiguousarray(xf[b]).astype(ml_dtypes.bfloat16),
            "wproj": wproj.astype(ml_dtypes.bfloat16),
            "wot": np.ascontiguousarray(
                np.tile(wot, (4, 1))).astype(ml_dtypes.bfloat16),
            "ub": ub,
            "vb": vb,
            "gamma": gamma,
        })
    return in_maps


def unshard_output(results, x_shape, bo):
    B, C_, H, W = x_shape
    out = np.empty((B, C_, H * W), dtype=np.float32)
    for b in range(B):
        out[b] = results[2 * b]["out"] + results[2 * b + 1]["out"] \
            + bo[:, None]
    return out.reshape(B, C_, H, W)


_NC_CACHE = []


def run(inputs, trace=False, tmpdir=None):
    """Run on 8 cores; returns (full_output, exec_time_ns_or_None)."""
    x = np.asarray(inputs["x"], dtype=np.float32)
    Wp = np.asarray(inputs["Wp"], dtype=np.float32)
    bp = np.asarray(inputs["bp"], dtype=np.float32)
    Wo = np.asarray(inputs["Wo"], dtype=np.float32)
    bo = np.asarray(inputs["bo"], dtype=np.float32)

    if not _NC_CACHE:
        _NC_CACHE.append(build_kernel())
    nc = _NC_CACHE[0]

    in_maps = shard_inputs(x, Wp, bp, Wo)
    kwargs = {}
    if trace:
        import tempfile
        kwargs = dict(trace=True,
                      tmpdir=tmpdir or tempfile.mkdtemp(prefix="attn_tr_"))
    res = bass_utils.run_bass_kernel_spmd(nc, in_maps,
                                          core_ids=list(range(8)), **kwargs)
    out = unshard_output(res.results, x.shape, bo)
    return out, res.exec_time_ns


def kernel(x, Wp, bp, Wo, bo):
    out, _ = run({"x": x, "Wp": Wp, "bp": bp, "Wo": Wo, "bo": bo})
    return out
